# revision 9
# baseline (speedup 1.0000x reference)
"""Brute-force KNN retrieval (B=512 queries, N=500000 candidates, D=128, top-K)
on 8 Trainium2 NeuronCores.

Strategy: candidates sharded along N across the 8 cores, queries replicated.
Per core, per 2048-candidate PSUM chunk:
  - PE computes bf16 scores (fp32 PSUM).
  - A pairwise-max tournament reduces the chunk to 128 "class maxima"
    (class = position mod 128; 16 members per class). Round 1 fuses the
    fp32->fp16 convert: tensor_max(fp16_out, psum[:1024], psum[1024:]) -- a
    single DVE op both converts and halves. Rounds 2-4 run on fp16 where
    tensor_tensor hits the DVE 2x_1p fast mode. For a tunable fraction of
    chunks the convert instead runs on the ACT engine (full fp32->fp16 copy)
    so both engines share the per-element work.
  - Every 4 chunks (a "window" of 512 classes), ACT packs the class maxima
    as fp16(max*0.25+14.25) into the HIGH int16 lanes of an iota-carrying
    fp32 tile and DVE max8 extracts the top-8 classes (values AND class ids
    in one pass).
Survivors: 8 windows x 8 = 64 classes per (row, core), 512 per row. The host
ranks them by packed value, exactly rescores the 16 members of each of the
top R classes per row in fp32, and emits the exact global top-K
(ties -> lower index, like lax.top_k). A class that survives has ALL its
members rescored, so multiple top-100 members in one class are all found.
"""

import sys

for _p in ("/opt/trn_rl_repo",):
    if _p not in sys.path:
        sys.path.insert(0, _p)

import numpy as np

B, N, D = 512, 500000, 128
N_CORES = 8
SHARD = N // N_CORES          # 62500 candidates per core
PCHUNK = 2048                 # PSUM tile width (4 banks)
NCHUNK = -(-SHARD // PCHUNK)  # 31
PADN = PCHUNK * NCHUNK        # 63488 (padded shard width)
NSUB = PCHUNK // 512          # 4 matmuls per PSUM tile
MTILES = B // 128             # 4 query tiles
W = 128                       # classes per chunk (class = pos mod 128)
G = PCHUNK // W               # 16 members per class
WIN_CHUNKS = 4                # chunks per level-2 window
NWIN = -(-NCHUNK // WIN_CHUNKS)   # 8 windows (last = 3 chunks, 384 classes)
SURV_PER_CORE = NWIN * 8      # 64 surviving classes per (row, core)
R_CLASSES = 160               # host exactly rescores top-R classes per row
PACK_SCALE = 0.25             # packed fp16 = max*0.25 + 14.25 (positive)
PACK_BIAS = 14.25

# Per-(chunk, mtile) iteration schedule, cycled. Each entry is a path code
# plus the engine for tournament rounds 2-4 ('d' = DVE, 'p' = gpsimd/Pool):
#   A = ACT full fp32->fp16 convert, DVE fp16 round 1
#   D = DVE fused convert+round1 from PSUM (dual-PSUM tensor_tensor)
#   H = ACT converts high half; DVE round 1 = max(PSUM low half, cvt f16)
#   S = DVE tensor_scalar fp32->fp16 convert, DVE fp16 round 1
#   M = DMA copies the fp32 PSUM tile to SBUF (DMA engines are otherwise
#       mostly idle); DVE round 1 fuses convert+halve on the SBUF fp32 pair
import os as _os

SCHED = _os.environ.get("KNN_SCHED", "Ad").split(",")

_NC_CACHE = {}


def _build_nc():
    import concourse.bacc as bacc
    import concourse.tile as tile
    import concourse.mybir as mybir

    f32 = mybir.dt.float32
    f16 = mybir.dt.float16
    u16 = mybir.dt.uint16
    bf16 = mybir.dt.bfloat16
    MAX = mybir.AluOpType.max

    nc = bacc.Bacc(
        "TRN2", target_bir_lowering=False, debug=False, num_devices=N_CORES
    )
    qT = nc.dram_tensor("qT", [D, B], bf16, kind="ExternalInput")
    cT = nc.dram_tensor("cT", [D, PADN], bf16, kind="ExternalInput")
    surv = nc.dram_tensor("surv", [B, SURV_PER_CORE], f32, kind="ExternalOutput")

    with tile.TileContext(nc) as tc:
        with (
            tc.tile_pool(name="q", bufs=1) as qp,
            tc.tile_pool(name="c", bufs=4) as cp,
            tc.tile_pool(name="ps", bufs=2, space="PSUM") as pp,
            tc.tile_pool(name="cvt", bufs=5) as vp,
            tc.tile_pool(name="h1", bufs=4) as h1p,
            tc.tile_pool(name="h2", bufs=4) as h2p,
            tc.tile_pool(name="h3", bufs=4) as h3p,
            tc.tile_pool(name="m32", bufs=2) as mp,
            tc.tile_pool(name="cls", bufs=1) as clp,
            tc.tile_pool(name="pk", bufs=1) as sp,
            tc.tile_pool(name="out", bufs=1) as op,
        ):
            qt = qp.tile([128, B], bf16)
            nc.sync.dma_start(qt[:], qT.ap())

            # survivors accumulate here, DMA'd out at the end
            sv = [
                op.tile([128, SURV_PER_CORE], f32, name=f"sv{m}", tag=f"sv{m}")
                for m in range(MTILES)
            ]
            # window class-max accumulation tiles: [128, 512] fp16, one live
            # window per m-tile (double-buffered across windows)
            cls = [
                [
                    clp.tile([128, W * WIN_CHUNKS], f16,
                             name=f"cls{m}_{j}", tag=f"cls{m}_{j}")
                    for j in range(2)
                ]
                for m in range(MTILES)
            ]
            # iota-carrying packed tiles for level-2 (low u16 lane = class id
            # within window, written once; ACT rewrites only the high lane)
            packed = [
                sp.tile([128, W * WIN_CHUNKS], f32, name=f"pk{j}", tag=f"pk{j}")
                for j in range(3)
            ]
            for j in range(3):
                lo = packed[j][:].bitcast(u16).rearrange(
                    "p (n two) -> p n two", two=2
                )[:, :, 0]
                nc.gpsimd.iota(lo, pattern=[[1, W * WIN_CHUNKS]], base=0,
                               channel_multiplier=0)

            it = 0          # (chunk, mtile) iteration counter
            npk = 0         # packed-tile rotation counter
            for c in range(NCHUNK):
                ct = cp.tile([128, PCHUNK], bf16, name=f"ct{c}", tag="ct")
                nc.sync.dma_start(ct[:], cT.ap()[:, c * PCHUNK:(c + 1) * PCHUNK])
                w = c // WIN_CHUNKS
                cw = c % WIN_CHUNKS
                wlen = min(WIN_CHUNKS, NCHUNK - w * WIN_CHUNKS)  # chunks in win
                for m in range(MTILES):
                    ps = pp.tile([128, PCHUNK], f32, name=f"ps{c}_{m}", tag="ps")
                    for s in range(NSUB):
                        nc.tensor.matmul(
                            ps[:, s * 512:(s + 1) * 512],
                            qt[:, m * 128:(m + 1) * 128],
                            ct[:, s * 512:(s + 1) * 512],
                            start=True,
                            stop=True,
                        )
                    code = SCHED[it % len(SCHED)]
                    path, reng = code[0], code[1]
                    h1 = h1p.tile([128, 1024], f16, name=f"h1_{it}", tag="h1")
                    if path == "A":
                        # ACT converts the whole chunk; DVE r1 all-fp16 (2x)
                        cv = vp.tile([128, PCHUNK], f16, name=f"cv{it}", tag="cv")
                        nc.scalar.activation(
                            cv[:], ps[:], mybir.ActivationFunctionType.Copy,
                            bias=0.0, scale=1.0,
                        )
                        nc.vector.tensor_tensor(
                            h1[:], cv[:, 0:1024], cv[:, 1024:2048], MAX)
                    elif path == "H":
                        # ACT converts the high half; DVE r1 reads PSUM low
                        # half + converted f16 (one PSUM operand only)
                        cv = vp.tile([128, 1024], f16, name=f"cv{it}", tag="cv")
                        nc.scalar.activation(
                            cv[:], ps[:, 1024:2048],
                            mybir.ActivationFunctionType.Copy,
                            bias=0.0, scale=1.0,
                        )
                        nc.vector.tensor_tensor(
                            h1[:], ps[:, 0:1024], cv[:], MAX)
                    elif path == "M":
                        # DMA drains PSUM to SBUF; DVE fuses convert+round 1
                        sb32 = mp.tile([128, PCHUNK], f32,
                                       name=f"m32_{it}", tag="m32")
                        nc.sync.dma_start(sb32[:], ps[:])
                        nc.vector.tensor_tensor(
                            h1[:], sb32[:, 0:1024], sb32[:, 1024:2048], MAX)
                    else:  # "S": DVE converts, DVE r1 fp16
                        cv = vp.tile([128, PCHUNK], f16, name=f"cv{it}", tag="cv")
                        nc.vector.tensor_scalar(
                            cv[:], ps[:], 0.0, None, mybir.AluOpType.add)
                        nc.vector.tensor_tensor(
                            h1[:], cv[:, 0:1024], cv[:, 1024:2048], MAX)
                    rv = nc.gpsimd if reng == "p" else nc.vector
                    h2 = h2p.tile([128, 512], f16, name=f"h2_{it}", tag="h2")
                    rv.tensor_tensor(
                        h2[:], h1[:, 0:512], h1[:, 512:1024], MAX)
                    h3 = h3p.tile([128, 256], f16, name=f"h3_{it}", tag="h3")
                    rv.tensor_tensor(
                        h3[:], h2[:, 0:256], h2[:, 256:512], MAX)
                    cw_t = cls[m][w % 2]
                    rv.tensor_tensor(
                        cw_t[:, cw * W:(cw + 1) * W],
                        h3[:, 0:128], h3[:, 128:256], MAX)
                    it += 1

                    if cw == wlen - 1:
                        # level-2: pack this window's class maxima and keep
                        # the top-8 classes (value|id) per row
                        pk = packed[npk % 3]
                        npk += 1
                        wid = wlen * W
                        hi = pk[:].bitcast(f16).rearrange(
                            "p (n two) -> p n two", two=2
                        )[:, 0:wid, 1]
                        nc.scalar.activation(
                            hi, cw_t[:, 0:wid],
                            mybir.ActivationFunctionType.Copy,
                            bias=PACK_BIAS, scale=PACK_SCALE,
                        )
                        nc.vector.max(
                            sv[m][:, w * 8:(w + 1) * 8], pk[:, 0:wid])

            for m in range(MTILES):
                nc.sync.dma_start(
                    surv.ap()[m * 128:(m + 1) * 128, :], sv[m][:])

    nc.compile()
    return nc


def _get_nc():
    if "nc" not in _NC_CACHE:
        _NC_CACHE["nc"] = _build_nc()
    return _NC_CACHE["nc"]


def _make_in_maps(queries, candidates):
    import ml_dtypes

    bf = ml_dtypes.bfloat16
    q = np.asarray(queries, dtype=np.float32)
    cand = np.asarray(candidates, dtype=np.float32)
    qTh = np.ascontiguousarray(q.T.astype(bf))  # [D, B] bf16
    in_maps = []
    for i in range(N_CORES):
        cTi = np.zeros((D, PADN), dtype=bf)
        cTi[:, :SHARD] = cand[i * SHARD:(i + 1) * SHARD].T.astype(bf)
        in_maps.append({"qT": qTh, "cT": cTi})
    return in_maps


def _run_device(in_maps, trace=False):
    from concourse import bass_utils

    nc = _get_nc()
    return bass_utils.run_bass_kernel_spmd(
        nc, in_maps, core_ids=list(range(N_CORES)), trace=trace
    )


def _merge(results, queries, candidates, identifiers, num_candidates):
    K = int(num_candidates)
    q = np.asarray(queries, dtype=np.float32)
    cand = np.asarray(candidates, dtype=np.float32)

    # Decode survivors: per (row, core) 64 slots = 8 windows x top-8.
    # Packed u32 = [fp16(max*0.25+14.25) | u16 class-id-in-window]; the u32
    # itself orders by (quantized class max, class id).
    slot_win = np.repeat(np.arange(NWIN), 8)            # [64] window of slot
    all_u = []
    all_cls = []                                        # global class id
    for i in range(N_CORES):
        u = np.asarray(results[i]["surv"]).view(np.uint32)   # [B, 64]
        cid = u & 0xFFFF                                     # class in window
        wchunk = slot_win[None, :] * WIN_CHUNKS              # window base chunk
        chunk = wchunk + (cid >> 7)                          # global chunk
        cls_local = chunk * W + (cid & (W - 1))              # class in shard
        all_u.append(u)
        all_cls.append(i * (NCHUNK * W) + cls_local)
    ucat = np.concatenate(all_u, axis=1)       # [B, 512]
    ccat = np.concatenate(all_cls, axis=1)     # [B, 512] global class ids

    # top-R classes per row by packed rank
    nres = min(R_CLASSES, ucat.shape[1])
    part = np.argpartition(ucat, ucat.shape[1] - nres, axis=1)[:, -nres:]
    rows = np.arange(B)[:, None]
    csel = np.take_along_axis(ccat, part, axis=1)        # [B, R]

    # expand classes -> member candidate indices
    gcls = csel                                           # global class id
    core = gcls // (NCHUNK * W)
    loc = gcls % (NCHUNK * W)
    chunk = loc // W
    pos0 = chunk * PCHUNK + (loc % W)                     # first member pos
    # members: pos0 + 128*j  (j in 0..15), pad-clipped
    mem = pos0[:, :, None] + W * np.arange(G)[None, None, :]   # [B, R, G]
    valid = mem < SHARD
    gidx = core[:, :, None] * SHARD + np.minimum(mem, SHARD - 1)
    gidx = gidx.reshape(B, -1)                            # [B, R*G]
    valid = valid.reshape(B, -1)

    # exact fp32 rescore (chunked over rows to bound the gather size)
    K_out_vals = np.empty((B, K), dtype=np.float32)
    K_out_gidx = np.empty((B, K), dtype=np.int64)
    step = 128
    NEG = np.float32(-3e38)
    for r0 in range(0, B, step):
        r1 = r0 + step
        gs = gidx[r0:r1]
        cs = cand[gs]                                     # [step, R*G, D]
        vs = np.einsum("bjd,bd->bj", cs, q[r0:r1], dtype=np.float32)
        vs = np.where(valid[r0:r1], vs, NEG)
        # dedupe safety: identical gidx entries (pad-clipped) rank together;
        # ties -> lower global index, like lax.top_k
        order = np.lexsort((gs, -vs), axis=-1)[:, :K]
        K_out_vals[r0:r1] = np.take_along_axis(vs, order, axis=1)
        K_out_gidx[r0:r1] = np.take_along_axis(gs, order, axis=1)

    ids = np.asarray(identifiers)
    out_ids = np.take(ids, K_out_gidx, axis=0)
    return K_out_vals, out_ids


def kernel(queries, candidates, identifiers, num_candidates):
    in_maps = _make_in_maps(queries, candidates)
    res = _run_device(in_maps, trace=False)
    return _merge(res.results, queries, candidates, identifiers, num_candidates)


# revision 12
# speedup vs baseline: 1.0322x; 1.0322x over previous
"""Brute-force KNN retrieval (B=512 queries, N=500000 candidates, D=128, top-K)
on 8 Trainium2 NeuronCores.

Strategy: candidates sharded along N across the 8 cores, queries replicated.
Per core, per 2048-candidate PSUM chunk:
  - PE computes bf16 scores (fp32 PSUM).
  - A pairwise-max tournament reduces the chunk to 128 "class maxima"
    (class = position mod 128; 16 members per class). The schedule mixes two
    legal leave-PSUM paths (a DVE instruction may read at most ONE operand
    from PSUM): "A" = ACT converts the whole chunk fp32->fp16, DVE round 1
    on fp16 (2x_1p fast mode); "H" = ACT converts only the high half and
    DVE round 1 fuses convert+halve as max(PSUM fp32 low half, converted
    fp16). Rounds 2-4 run on fp16 at DVE 2x. The A:H ratio balances the
    ACT (0.833 ns/elem) and DVE (1.042 ns/elem) engines.
  - Every 4 chunks (a "window" of 512 classes), ACT packs the class maxima
    as fp16(max*0.25+14.25) into the HIGH int16 lanes of an iota-carrying
    fp32 tile and DVE max8 extracts the top-8 classes (values AND class ids
    in one pass).
Survivors: 8 windows x 8 = 64 classes per (row, core), 512 per row. The host
ranks them by packed value, exactly rescores the 16 members of each of the
top R classes per row in fp32, and emits the exact global top-K
(ties -> lower index, like lax.top_k). A class that survives has ALL its
members rescored, so multiple top-100 members in one class are all found.
"""

import sys

for _p in ("/opt/trn_rl_repo",):
    if _p not in sys.path:
        sys.path.insert(0, _p)

import numpy as np

B, N, D = 512, 500000, 128
N_CORES = 8
SHARD = N // N_CORES          # 62500 candidates per core
PCHUNK = 2048                 # PSUM tile width (4 banks)
NCHUNK = -(-SHARD // PCHUNK)  # 31
PADN = PCHUNK * NCHUNK        # 63488 (padded shard width)
NSUB = PCHUNK // 512          # 4 matmuls per PSUM tile
MTILES = B // 128             # 4 query tiles
W = 128                       # classes per chunk (class = pos mod 128)
G = PCHUNK // W               # 16 members per class
WIN_CHUNKS = 4                # chunks per level-2 window
NWIN = -(-NCHUNK // WIN_CHUNKS)   # 8 windows (last = 3 chunks, 384 classes)
SURV_PER_CORE = NWIN * 8      # 64 surviving classes per (row, core)
R_CLASSES = 160               # host exactly rescores top-R classes per row
PACK_SCALE = 0.25             # packed fp16 = max*0.25 + 14.25 (positive)
PACK_BIAS = 14.25

# Per-(chunk, mtile) iteration schedule, cycled. Each entry is a path code
# plus the engine for tournament rounds 2-4 ('d' = DVE, 'p' = gpsimd/Pool):
#   A = ACT full fp32->fp16 convert, DVE fp16 round 1
#   D = DVE fused convert+round1 from PSUM (dual-PSUM tensor_tensor)
#   H = ACT converts high half; DVE round 1 = max(PSUM low half, cvt f16)
#   S = DVE tensor_scalar fp32->fp16 convert, DVE fp16 round 1
#   M = DMA copies the fp32 PSUM tile to SBUF (DMA engines are otherwise
#       mostly idle); DVE round 1 fuses convert+halve on the SBUF fp32 pair
import os as _os

SCHED = _os.environ.get(
    "KNN_SCHED", "Ad,Hd,Ad,Hd,Ad,Hd,Ad,Hd,Ad"
).split(",")

_NC_CACHE = {}


def _build_nc():
    import concourse.bacc as bacc
    import concourse.tile as tile
    import concourse.mybir as mybir

    f32 = mybir.dt.float32
    f16 = mybir.dt.float16
    u16 = mybir.dt.uint16
    bf16 = mybir.dt.bfloat16
    MAX = mybir.AluOpType.max

    nc = bacc.Bacc(
        "TRN2", target_bir_lowering=False, debug=False, num_devices=N_CORES
    )
    qT = nc.dram_tensor("qT", [D, B], bf16, kind="ExternalInput")
    cT = nc.dram_tensor("cT", [D, PADN], bf16, kind="ExternalInput")
    surv = nc.dram_tensor("surv", [B, SURV_PER_CORE], f32, kind="ExternalOutput")

    with tile.TileContext(nc) as tc:
        with (
            tc.tile_pool(name="q", bufs=1) as qp,
            tc.tile_pool(name="c", bufs=4) as cp,
            tc.tile_pool(name="ps", bufs=2, space="PSUM") as pp,
            tc.tile_pool(name="cvt", bufs=3) as vp,
            tc.tile_pool(name="h1", bufs=3) as h1p,
            tc.tile_pool(name="h2", bufs=3) as h2p,
            tc.tile_pool(name="h3", bufs=3) as h3p,
            tc.tile_pool(name="m32", bufs=2) as mp,
            tc.tile_pool(name="cls", bufs=1) as clp,
            tc.tile_pool(name="pk", bufs=1) as sp,
            tc.tile_pool(name="out", bufs=1) as op,
        ):
            qt = qp.tile([128, B], bf16)
            nc.sync.dma_start(qt[:], qT.ap())

            # survivors accumulate here, DMA'd out at the end
            sv = [
                op.tile([128, SURV_PER_CORE], f32, name=f"sv{m}", tag=f"sv{m}")
                for m in range(MTILES)
            ]
            # window class-max accumulation tiles: [128, 512] fp16, one live
            # window per m-tile (double-buffered across windows)
            cls = [
                [
                    clp.tile([128, W * WIN_CHUNKS], f16,
                             name=f"cls{m}_{j}", tag=f"cls{m}_{j}")
                    for j in range(2)
                ]
                for m in range(MTILES)
            ]
            # iota-carrying packed tiles for level-2 (low u16 lane = class id
            # within window, written once; ACT rewrites only the high lane)
            packed = [
                sp.tile([128, W * WIN_CHUNKS], f32, name=f"pk{j}", tag=f"pk{j}")
                for j in range(3)
            ]
            for j in range(3):
                lo = packed[j][:].bitcast(u16).rearrange(
                    "p (n two) -> p n two", two=2
                )[:, :, 0]
                nc.gpsimd.iota(lo, pattern=[[1, W * WIN_CHUNKS]], base=0,
                               channel_multiplier=0)

            it = 0          # (chunk, mtile) iteration counter
            npk = 0         # packed-tile rotation counter
            for c in range(NCHUNK):
                ct = cp.tile([128, PCHUNK], bf16, name=f"ct{c}", tag="ct")
                nc.sync.dma_start(ct[:], cT.ap()[:, c * PCHUNK:(c + 1) * PCHUNK])
                w = c // WIN_CHUNKS
                cw = c % WIN_CHUNKS
                wlen = min(WIN_CHUNKS, NCHUNK - w * WIN_CHUNKS)  # chunks in win
                for m in range(MTILES):
                    ps = pp.tile([128, PCHUNK], f32, name=f"ps{c}_{m}", tag="ps")
                    for s in range(NSUB):
                        nc.tensor.matmul(
                            ps[:, s * 512:(s + 1) * 512],
                            qt[:, m * 128:(m + 1) * 128],
                            ct[:, s * 512:(s + 1) * 512],
                            start=True,
                            stop=True,
                        )
                    code = SCHED[it % len(SCHED)]
                    path, reng = code[0], code[1]
                    h1 = h1p.tile([128, 1024], f16, name=f"h1_{it}", tag="h1")
                    if path == "A":
                        # ACT converts the whole chunk; DVE r1 all-fp16 (2x)
                        cv = vp.tile([128, PCHUNK], f16, name=f"cv{it}", tag="cv")
                        nc.scalar.activation(
                            cv[:], ps[:], mybir.ActivationFunctionType.Copy,
                            bias=0.0, scale=1.0,
                        )
                        nc.vector.tensor_tensor(
                            h1[:], cv[:, 0:1024], cv[:, 1024:2048], MAX)
                    elif path == "H":
                        # ACT converts the high half; DVE r1 reads PSUM low
                        # half + converted f16 (one PSUM operand only)
                        cv = vp.tile([128, 1024], f16, name=f"cv{it}", tag="cv")
                        nc.scalar.activation(
                            cv[:], ps[:, 1024:2048],
                            mybir.ActivationFunctionType.Copy,
                            bias=0.0, scale=1.0,
                        )
                        nc.vector.tensor_tensor(
                            h1[:], ps[:, 0:1024], cv[:], MAX)
                    elif path == "M":
                        # DMA drains PSUM to SBUF; DVE fuses convert+round 1
                        sb32 = mp.tile([128, PCHUNK], f32,
                                       name=f"m32_{it}", tag="m32")
                        nc.sync.dma_start(sb32[:], ps[:])
                        nc.vector.tensor_tensor(
                            h1[:], sb32[:, 0:1024], sb32[:, 1024:2048], MAX)
                    else:  # "S": DVE converts, DVE r1 fp16
                        cv = vp.tile([128, PCHUNK], f16, name=f"cv{it}", tag="cv")
                        nc.vector.tensor_scalar(
                            cv[:], ps[:], 0.0, None, mybir.AluOpType.add)
                        nc.vector.tensor_tensor(
                            h1[:], cv[:, 0:1024], cv[:, 1024:2048], MAX)
                    rv = nc.gpsimd if reng == "p" else nc.vector
                    h2 = h2p.tile([128, 512], f16, name=f"h2_{it}", tag="h2")
                    rv.tensor_tensor(
                        h2[:], h1[:, 0:512], h1[:, 512:1024], MAX)
                    h3 = h3p.tile([128, 256], f16, name=f"h3_{it}", tag="h3")
                    rv.tensor_tensor(
                        h3[:], h2[:, 0:256], h2[:, 256:512], MAX)
                    cw_t = cls[m][w % 2]
                    rv.tensor_tensor(
                        cw_t[:, cw * W:(cw + 1) * W],
                        h3[:, 0:128], h3[:, 128:256], MAX)
                    it += 1

                    if cw == wlen - 1:
                        # level-2: pack this window's class maxima and keep
                        # the top-8 classes (value|id) per row
                        pk = packed[npk % 3]
                        npk += 1
                        wid = wlen * W
                        hi = pk[:].bitcast(f16).rearrange(
                            "p (n two) -> p n two", two=2
                        )[:, 0:wid, 1]
                        nc.scalar.activation(
                            hi, cw_t[:, 0:wid],
                            mybir.ActivationFunctionType.Copy,
                            bias=PACK_BIAS, scale=PACK_SCALE,
                        )
                        nc.vector.max(
                            sv[m][:, w * 8:(w + 1) * 8], pk[:, 0:wid])

            for m in range(MTILES):
                nc.sync.dma_start(
                    surv.ap()[m * 128:(m + 1) * 128, :], sv[m][:])

    nc.compile()
    return nc


def _get_nc():
    if "nc" not in _NC_CACHE:
        _NC_CACHE["nc"] = _build_nc()
    return _NC_CACHE["nc"]


def _make_in_maps(queries, candidates):
    import ml_dtypes

    bf = ml_dtypes.bfloat16
    q = np.asarray(queries, dtype=np.float32)
    cand = np.asarray(candidates, dtype=np.float32)
    qTh = np.ascontiguousarray(q.T.astype(bf))  # [D, B] bf16
    in_maps = []
    for i in range(N_CORES):
        cTi = np.zeros((D, PADN), dtype=bf)
        cTi[:, :SHARD] = cand[i * SHARD:(i + 1) * SHARD].T.astype(bf)
        in_maps.append({"qT": qTh, "cT": cTi})
    return in_maps


def _run_device(in_maps, trace=False):
    from concourse import bass_utils

    nc = _get_nc()
    return bass_utils.run_bass_kernel_spmd(
        nc, in_maps, core_ids=list(range(N_CORES)), trace=trace
    )


def _merge(results, queries, candidates, identifiers, num_candidates):
    K = int(num_candidates)
    q = np.asarray(queries, dtype=np.float32)
    cand = np.asarray(candidates, dtype=np.float32)

    # Decode survivors: per (row, core) 64 slots = 8 windows x top-8.
    # Packed u32 = [fp16(max*0.25+14.25) | u16 class-id-in-window]; the u32
    # itself orders by (quantized class max, class id).
    slot_win = np.repeat(np.arange(NWIN), 8)            # [64] window of slot
    all_u = []
    all_cls = []                                        # global class id
    for i in range(N_CORES):
        u = np.asarray(results[i]["surv"]).view(np.uint32)   # [B, 64]
        cid = u & 0xFFFF                                     # class in window
        wchunk = slot_win[None, :] * WIN_CHUNKS              # window base chunk
        chunk = wchunk + (cid >> 7)                          # global chunk
        cls_local = chunk * W + (cid & (W - 1))              # class in shard
        all_u.append(u)
        all_cls.append(i * (NCHUNK * W) + cls_local)
    ucat = np.concatenate(all_u, axis=1)       # [B, 512]
    ccat = np.concatenate(all_cls, axis=1)     # [B, 512] global class ids

    # top-R classes per row by packed rank
    nres = min(R_CLASSES, ucat.shape[1])
    part = np.argpartition(ucat, ucat.shape[1] - nres, axis=1)[:, -nres:]
    rows = np.arange(B)[:, None]
    csel = np.take_along_axis(ccat, part, axis=1)        # [B, R]

    # expand classes -> member candidate indices
    gcls = csel                                           # global class id
    core = gcls // (NCHUNK * W)
    loc = gcls % (NCHUNK * W)
    chunk = loc // W
    pos0 = chunk * PCHUNK + (loc % W)                     # first member pos
    # members: pos0 + 128*j  (j in 0..15), pad-clipped
    mem = pos0[:, :, None] + W * np.arange(G)[None, None, :]   # [B, R, G]
    valid = mem < SHARD
    gidx = core[:, :, None] * SHARD + np.minimum(mem, SHARD - 1)
    gidx = gidx.reshape(B, -1)                            # [B, R*G]
    valid = valid.reshape(B, -1)

    # exact fp32 rescore (chunked over rows to bound the gather size)
    K_out_vals = np.empty((B, K), dtype=np.float32)
    K_out_gidx = np.empty((B, K), dtype=np.int64)
    step = 128
    NEG = np.float32(-3e38)
    for r0 in range(0, B, step):
        r1 = r0 + step
        gs = gidx[r0:r1]
        cs = cand[gs]                                     # [step, R*G, D]
        vs = np.einsum("bjd,bd->bj", cs, q[r0:r1], dtype=np.float32)
        vs = np.where(valid[r0:r1], vs, NEG)
        # dedupe safety: identical gidx entries (pad-clipped) rank together;
        # ties -> lower global index, like lax.top_k
        order = np.lexsort((gs, -vs), axis=-1)[:, :K]
        K_out_vals[r0:r1] = np.take_along_axis(vs, order, axis=1)
        K_out_gidx[r0:r1] = np.take_along_axis(gs, order, axis=1)

    ids = np.asarray(identifiers)
    out_ids = np.take(ids, K_out_gidx, axis=0)
    return K_out_vals, out_ids


def kernel(queries, candidates, identifiers, num_candidates):
    in_maps = _make_in_maps(queries, candidates)
    res = _run_device(in_maps, trace=False)
    return _merge(res.results, queries, candidates, identifiers, num_candidates)
